# revision 8
# baseline (speedup 1.0000x reference)
"""Cross-attention layer (B=2, L=2048, D=1024, 16 heads) on 8 TRN2 NeuronCores.

Sharding: zero-communication data parallel over (batch x query-row-quarter).
Core c handles b = c//4, query rows [512*(c%4) : 512*(c%4)+512], all 16 heads.
K/V projections for the core's batch are computed on-core (replicated 4x of
that batch's K/V work); everything else is perfectly sharded.

Device pipeline per core (all matmuls in float32r, full PE rate):
  1. QK^T projections as transposed activations (weights are lhsT, the
     host supplies query^T / key_value^T so no on-device transposes).
  2. Scores S^T[kv, q] per head via head-pair-packed tile_position matmuls
     (contraction = head_dim 64).
  3. exp on ScalarE straight out of 2-bank PSUM groups with scale=1/8
     (softmax max-subtraction skipped: |scores/8| < ~2 for this model).
  4. ctx^T via ones-augmented V (M=65): row 64 accumulates the softmax
     denominator for free.
  5. Normalize by the denominator (reciprocal + PE broadcast), add bv.
  6. Output projection from ctx^T tiles, residual (query+bo pre-added on
     host), LayerNorm, gamma/beta.
"""

import numpy as np

import concourse.bass as bass
import concourse.mybir as mybir
import concourse.tile as tile
from concourse import bacc
from concourse.bass_utils import run_bass_kernel_spmd

dt = mybir.dt
AF = mybir.ActivationFunctionType
ALU = mybir.AluOpType

P = 128
B, LQ, LKV = 2, 2048, 2048
DQ, DKV, HID, NH = 1024, 1024, 1024, 16
HD = HID // NH                     # 64
EPS = 1e-5
N_CORES = 8
RQ = LQ * B // N_CORES             # 512 query rows per core
KV_T = LKV // P                    # 16 kv tiles
DPO = DQ // P                      # 8 contraction tiles
N_PAIR = NH // 2                   # 8 head pairs
N_QUART = NH // 4                  # 4 head quartets (V granularity)


def build_nc():
    nc = bacc.Bacc("TRN2", target_bir_lowering=False, debug=False,
                   num_devices=N_CORES)

    f32r, f32 = dt.float32r, dt.float32
    qT_d = nc.dram_tensor("qT", [DQ, RQ], f32r, kind="ExternalInput")
    kvT_d = nc.dram_tensor("kvT", [DKV, LKV], f32r, kind="ExternalInput")
    xq_d = nc.dram_tensor("xq", [RQ, HID], f32, kind="ExternalInput")
    wq_d = nc.dram_tensor("wq", [DQ, HID], f32r, kind="ExternalInput")
    wk_d = nc.dram_tensor("wk", [DKV, HID], f32r, kind="ExternalInput")
    wv_d = nc.dram_tensor("wv", [DKV, HID], f32r, kind="ExternalInput")
    wo_d = nc.dram_tensor("wo", [HID, DQ], f32r, kind="ExternalInput")
    bq_d = nc.dram_tensor("bq", [N_PAIR, P, 1], f32, kind="ExternalInput")
    bk_d = nc.dram_tensor("bk", [N_PAIR, P, 1], f32, kind="ExternalInput")
    bv_d = nc.dram_tensor("bv", [NH, HD, 1], f32, kind="ExternalInput")
    gam_d = nc.dram_tensor("gamma", [1, DQ], f32r, kind="ExternalInput")
    bet_d = nc.dram_tensor("beta", [1, DQ], f32r, kind="ExternalInput")
    out_d = nc.dram_tensor("out", [RQ, DQ], f32, kind="ExternalOutput")

    with tile.TileContext(nc) as tc:
        const_cm = tc.tile_pool(name="const", bufs=1)
        const = const_cm.__enter__()
        ones_col = const.tile([P, KV_T, 4, 1], f32)
        nc.vector.memset(ones_col[:], 1.0)
        g_row = const.tile([1, DQ], f32r)
        b_row = const.tile([1, DQ], f32r)
        nc.sync.dma_start(g_row[:], gam_d.ap())
        nc.sync.dma_start(b_row[:], bet_d.ap())
        # gamma/beta broadcast to all 128 partitions on GpSimd
        gb_bc = const.tile([P, 2, DQ], f32)
        for i, row in enumerate((g_row, b_row)):
            nc.gpsimd.partition_broadcast(gb_bc[:, i, :], row[:].bitcast(f32))

        # resident activations
        qT_sb = const.tile([P, DPO, RQ], f32r)
        kvT_sb = const.tile([P, DPO, LKV], f32r)
        for po in range(DPO):
            nc.sync.dma_start(
                qT_sb[:, po], qT_d.ap().rearrange("(po p) q -> po p q", p=P)[po])
            nc.sync.dma_start(
                kvT_sb[:, po], kvT_d.ap().rearrange("(po p) q -> po p q", p=P)[po])

        ctxT_sb = const.tile([P, N_PAIR, RQ], f32r)
        eps_t = const.tile([P, 1], f32)
        nc.vector.memset(eps_t[:], EPS)

        wq_r = wq_d.ap().rearrange("(po p) h -> p po h", p=P)
        wk_r = wk_d.ap().rearrange("(po p) h -> p po h", p=P)
        wv_r = wv_d.ap().rearrange("(po p) h -> p po h", p=P)

        with (
            tc.tile_pool(name="wpool", bufs=2) as wpool,
            tc.tile_pool(name="wvpool", bufs=1) as wvpool,
            tc.tile_pool(name="vpool", bufs=1) as vpool,
            tc.tile_pool(name="ktpool", bufs=2) as ktpool,
            tc.tile_pool(name="qtpool", bufs=2) as qtpool,
            tc.tile_pool(name="epool", bufs=2) as epool,
            tc.tile_pool(name="bpool", bufs=2) as bpool,
            tc.tile_pool(name="smpool", bufs=4) as smpool,
            tc.tile_pool(name="proj_ps", bufs=2, space="PSUM") as proj_ps,
            tc.tile_pool(name="sc_ps", bufs=2, space="PSUM") as sc_ps,
            tc.tile_pool(name="ctx_ps", bufs=2, space="PSUM") as ctx_ps,
        ):
            for hp in range(N_PAIR):
                # ---- V projection for this quartet (every other pair) ----
                if hp % 2 == 0:
                    qt = hp // 2
                    v_sb = vpool.tile([P, KV_T, 4, HD + 1], f32r, tag="v")
                    nc.vector.tensor_copy(v_sb[:, :, :, HD:], ones_col[:])
                    wv_blk = wvpool.tile([P, DPO, 4 * HD], f32r, tag="wv")
                    nc.sync.dma_start(
                        wv_blk[:],
                        wv_r[:, :, 4 * HD * qt: 4 * HD * (qt + 1)])
                    for t in range(KV_T):
                        ps_v = proj_ps.tile([P, 4 * HD], f32, tag="proj")
                        for po in range(DPO):
                            nc.tensor.matmul(
                                ps_v[:], kvT_sb[:, po, P * t:P * (t + 1)],
                                wv_blk[:, po], start=(po == 0), stop=(po == DPO - 1))
                        nc.vector.tensor_copy(
                            v_sb[:, t, :, :HD],
                            ps_v[:].rearrange("p (h d) -> p h d", h=4))

                # ---- Q^T for pair ----
                b_q = smpool.tile([P, 1], f32, tag="bias")
                nc.sync.dma_start(b_q[:], bq_d.ap()[hp])
                b_k = smpool.tile([P, 1], f32, tag="bias")
                nc.sync.dma_start(b_k[:], bk_d.ap()[hp])
                wq_blk = wpool.tile([P, DPO, P], f32r, tag="wq")
                nc.sync.dma_start(
                    wq_blk[:], wq_r[:, :, P * hp:P * (hp + 1)])
                wk_blk = wpool.tile([P, DPO, P], f32r, tag="wk")
                nc.sync.dma_start(
                    wk_blk[:], wk_r[:, :, P * hp:P * (hp + 1)])

                ps_q = proj_ps.tile([P, RQ], f32, tag="proj")
                for po in range(DPO):
                    nc.tensor.matmul(ps_q[:], wq_blk[:, po], qT_sb[:, po],
                                     start=(po == 0), stop=(po == DPO - 1))
                qt_pair = qtpool.tile([P, RQ], f32r, tag="qt")
                nc.scalar.activation(qt_pair[:], ps_q[:], AF.Identity, bias=b_q[:])

                # ---- K^T for pair (4 chunks of 512 kv cols) ----
                kt_pair = ktpool.tile([P, LKV], f32r, tag="kt")
                for c in range(4):
                    ps_k = proj_ps.tile([P, 512], f32, tag="proj")
                    for po in range(DPO):
                        nc.tensor.matmul(
                            ps_k[:], wk_blk[:, po],
                            kvT_sb[:, po, 512 * c:512 * (c + 1)],
                            start=(po == 0), stop=(po == DPO - 1))
                    nc.scalar.activation(kt_pair[:, 512 * c:512 * (c + 1)], ps_k[:],
                                         AF.Identity, bias=b_k[:])

                # ---- attention for the two heads of this pair ----
                ps_c = [ctx_ps.tile([HD + 1, RQ], f32, tag="ctx", name=f"ps_c{_h}")
                        for _h in range(2)]
                for g in range(KV_T // 2):
                    ps_s = [sc_ps.tile([P, 2, RQ], f32, tag="sc", name=f"ps_s{_h}")
                        for _h in range(2)]
                    for h in range(2):
                        lo, hi = HD * h, HD * (h + 1)
                        for t in range(2):
                            kv = 2 * g + t
                            nc.tensor.matmul(
                                ps_s[h][:, t], kt_pair[lo:hi, P * kv:P * (kv + 1)],
                                qt_pair[lo:hi, :], start=True, stop=True,
                                tile_position=(HD * h, 0))
                    for h in range(2):
                        e_t = epool.tile([P, 2, RQ], f32r, tag="e")
                        nc.scalar.activation(e_t[:], ps_s[h][:], AF.Exp,
                                             scale=1.0 / np.sqrt(HD))
                        for t in range(2):
                            kv = 2 * g + t
                            nc.tensor.matmul(
                                ps_c[h][:], v_sb[:, kv, (hp % 2) * 2 + h, :],
                                e_t[:, t], start=(g == 0 and t == 0),
                                stop=(g == KV_T // 2 - 1 and t == 1))

                # ---- normalize ctx^T, add bv, store into ctxT_sb ----
                for h in range(2):
                    den = smpool.tile([1, RQ], f32, tag="den")
                    nc.vector.tensor_copy(den[:], ps_c[h][HD:HD + 1, :])
                    rec = smpool.tile([1, RQ], f32, tag="rec")
                    nc.vector.reciprocal(rec[:], den[:])
                    rec_bc = bpool.tile([HD, RQ], f32, tag="recbc")
                    nc.gpsimd.partition_broadcast(rec_bc[:], rec[:].bitcast(f32))
                    b_v = smpool.tile([HD, 1], f32, tag="bv")
                    nc.sync.dma_start(b_v[:], bv_d.ap()[2 * hp + h])
                    tmp = bpool.tile([HD, RQ], f32, tag="nrm")
                    nc.vector.tensor_tensor(tmp[:], ps_c[h][:HD, :], rec_bc[:],
                                            op=ALU.mult)
                    nc.vector.tensor_scalar(
                        ctxT_sb[HD * h:HD * (h + 1), hp, :], tmp[:], b_v[:], None,
                        op0=ALU.add)

        # ---- output projection + residual + LayerNorm ----
        with (
            tc.tile_pool(name="wo_pool", bufs=1) as wo_pool,
            tc.tile_pool(name="opool", bufs=2) as opool,
            tc.tile_pool(name="ln_sm", bufs=4) as ln_sm,
            tc.tile_pool(name="out_ps", bufs=2, space="PSUM") as out_ps,
        ):
            wo_sb = wo_pool.tile([P, DPO, DQ], f32r)
            for po in range(DPO):
                nc.sync.dma_start(
                    wo_sb[:, po],
                    wo_d.ap().rearrange("(po p) e -> po p e", p=P)[po])
            for m in range(RQ // P):
                ps_o = out_ps.tile([P, 2, 512], f32, tag="o")
                for n in range(2):
                    for po in range(DPO):
                        nc.tensor.matmul(
                            ps_o[:, n], ctxT_sb[:, po, P * m:P * (m + 1)],
                            wo_sb[:, po, 512 * n:512 * (n + 1)],
                            start=(po == 0), stop=(po == DPO - 1))
                xq_t = opool.tile([P, 2, 512], f32, tag="xq")
                nc.sync.dma_start(
                    xq_t[:],
                    xq_d.ap().rearrange("(m p) (n f) -> m p n f", p=P, f=512)[m])
                x = opool.tile([P, 2, 512], f32, tag="x")
                nc.vector.tensor_tensor(x[:], ps_o[:], xq_t[:], op=ALU.add)
                xf = x[:].rearrange("p a b -> p (a b)")
                mu = ln_sm.tile([P, 1], f32, tag="mu")
                nc.vector.tensor_reduce(mu[:], xf, axis=mybir.AxisListType.X,
                                        op=ALU.add)
                xx = opool.tile([P, 2, 512], f32, tag="xx")
                nc.vector.tensor_tensor(xx[:], x[:], x[:], op=ALU.mult)
                m2 = ln_sm.tile([P, 1], f32, tag="m2")
                nc.vector.tensor_reduce(m2[:], xx[:].rearrange("p a b -> p (a b)"),
                                        axis=mybir.AxisListType.X, op=ALU.add)
                nc.vector.tensor_scalar(mu[:], mu[:], 1.0 / DQ, None, op0=ALU.mult)
                musq = ln_sm.tile([P, 1], f32, tag="musq")
                nc.vector.tensor_tensor(musq[:], mu[:], mu[:], op=ALU.mult)
                var = ln_sm.tile([P, 1], f32, tag="var")
                nc.vector.tensor_scalar(var[:], m2[:], 1.0 / DQ, None, op0=ALU.mult)
                nc.vector.tensor_tensor(var[:], var[:], musq[:], op=ALU.subtract)
                sd = ln_sm.tile([P, 1], f32, tag="sd")
                nc.scalar.activation(sd[:], var[:], AF.Sqrt, bias=eps_t[:])
                rstd = ln_sm.tile([P, 1], f32, tag="rstd")
                nc.vector.reciprocal(rstd[:], sd[:])
                y = opool.tile([P, 2, 512], f32, tag="xx")
                nc.vector.tensor_scalar(
                    y[:].rearrange("p a b -> p (a b)"), xf, mu[:], rstd[:],
                    op0=ALU.subtract, op1=ALU.mult)
                z = opool.tile([P, 2, 512], f32, tag="x")
                nc.vector.tensor_tensor(
                    z[:], y[:], gb_bc[:, 0].rearrange("p (a b) -> p a b", b=512),
                    op=ALU.mult)
                z2 = opool.tile([P, 2, 512], f32, tag="xx")
                nc.vector.tensor_tensor(
                    z2[:], z[:], gb_bc[:, 1].rearrange("p (a b) -> p a b", b=512),
                    op=ALU.add)
                nc.sync.dma_start(
                    out_d.ap().rearrange("(m p) (n f) -> m p n f", p=P, f=512)[m],
                    z2[:])
        const_cm.__exit__(None, None, None)

    nc.compile()
    return nc


_NC_CACHE = None


def _get_nc():
    global _NC_CACHE
    if _NC_CACHE is None:
        _NC_CACHE = build_nc()
    return _NC_CACHE


def kernel(query, key_value, Wq, bq, Wk, bk, Wv, bv, Wo, bo, ln_gamma, ln_beta):
    query = np.asarray(query, dtype=np.float32)
    key_value = np.asarray(key_value, dtype=np.float32)
    Wq = np.ascontiguousarray(np.asarray(Wq, np.float32))
    Wk = np.ascontiguousarray(np.asarray(Wk, np.float32))
    Wv = np.ascontiguousarray(np.asarray(Wv, np.float32))
    Wo = np.ascontiguousarray(np.asarray(Wo, np.float32))
    bq = np.asarray(bq, np.float32).reshape(N_PAIR, P, 1)
    bk = np.asarray(bk, np.float32).reshape(N_PAIR, P, 1)
    bv = np.asarray(bv, np.float32).reshape(NH, HD, 1)
    gam = np.asarray(ln_gamma, np.float32).reshape(1, DQ)
    bet = np.asarray(ln_beta, np.float32).reshape(1, DQ)
    bo = np.asarray(bo, np.float32)

    nc = _get_nc()
    kvT = [np.ascontiguousarray(key_value[b].T) for b in range(B)]
    in_maps = []
    for c in range(N_CORES):
        b, rq = divmod(c, N_CORES // B)
        rows = slice(RQ * rq, RQ * (rq + 1))
        in_maps.append({
            "qT": np.ascontiguousarray(query[b, rows].T),
            "kvT": kvT[b],
            "xq": np.ascontiguousarray(query[b, rows] + bo),
            "wq": Wq, "wk": Wk, "wv": Wv, "wo": Wo,
            "bq": bq, "bk": bk, "bv": bv,
            "gamma": gam, "beta": bet,
        })
    res = run_bass_kernel_spmd(nc, in_maps, list(range(N_CORES)))
    out = np.concatenate([r["out"] for r in res.results], axis=0)
    return out.reshape(B, LQ, DQ)


# revision 14
# speedup vs baseline: 1.1605x; 1.1605x over previous
"""Cross-attention layer (B=2, L=2048, D=1024, 16 heads) on 8 TRN2 NeuronCores.

Sharding: zero-communication data parallel over (batch x query-row-quarter).
Core c handles b = c//4, query rows [512*(c%4) : 512*(c%4)+512], all 16 heads.
K/V projections for the core's batch are computed on-core (replicated 4x of
that batch's K/V work); everything else is perfectly sharded.

Device pipeline per core (all matmuls in float32r, full PE rate):
  1. QK^T projections as transposed activations (weights are lhsT, the
     host supplies query^T / key_value^T so no on-device transposes).
  2. Scores S^T[kv, q] per head via head-pair-packed tile_position matmuls
     (contraction = head_dim 64).
  3. exp on ScalarE straight out of 2-bank PSUM groups with scale=1/8
     (softmax max-subtraction skipped: |scores/8| < ~2 for this model).
  4. ctx^T via ones-augmented V (M=65): row 64 accumulates the softmax
     denominator for free.
  5. Normalize by the denominator (reciprocal + PE broadcast), add bv.
  6. Output projection from ctx^T tiles, residual (query+bo pre-added on
     host), LayerNorm, gamma/beta.
"""

import numpy as np

import concourse.bass as bass
import concourse.mybir as mybir
import concourse.tile as tile
from concourse import bacc
from concourse.bass_utils import run_bass_kernel_spmd

dt = mybir.dt
AF = mybir.ActivationFunctionType
ALU = mybir.AluOpType

P = 128
B, LQ, LKV = 2, 2048, 2048
DQ, DKV, HID, NH = 1024, 1024, 1024, 16
HD = HID // NH                     # 64
EPS = 1e-5
N_CORES = 8
RQ = LQ * B // N_CORES             # 512 query rows per core
KV_T = LKV // P                    # 16 kv tiles
DPO = DQ // P                      # 8 contraction tiles
N_PAIR = NH // 2                   # 8 head pairs
N_QUART = NH // 4                  # 4 head quartets (V granularity)


def build_nc():
    nc = bacc.Bacc("TRN2", target_bir_lowering=False, debug=False,
                   num_devices=N_CORES)

    f32r, f32 = dt.float32r, dt.float32
    qT_d = nc.dram_tensor("qT", [DQ, RQ], f32r, kind="ExternalInput")
    kvT_d = nc.dram_tensor("kvT", [DKV, LKV], f32r, kind="ExternalInput")
    xq_d = nc.dram_tensor("xq", [RQ, HID], f32, kind="ExternalInput")
    wq_d = nc.dram_tensor("wq", [DQ, HID], f32r, kind="ExternalInput")
    wk_d = nc.dram_tensor("wk", [DKV, HID], f32r, kind="ExternalInput")
    wv_d = nc.dram_tensor("wv", [DKV, HID], f32r, kind="ExternalInput")
    wo_d = nc.dram_tensor("wo", [HID, DQ], f32r, kind="ExternalInput")
    bq_d = nc.dram_tensor("bq", [N_PAIR, P, 1], f32, kind="ExternalInput")
    bk_d = nc.dram_tensor("bk", [N_PAIR, P, 1], f32, kind="ExternalInput")
    bv_d = nc.dram_tensor("bv", [NH, HD, 1], f32, kind="ExternalInput")
    gam_d = nc.dram_tensor("gamma", [1, DQ], f32r, kind="ExternalInput")
    bet_d = nc.dram_tensor("beta", [1, DQ], f32r, kind="ExternalInput")
    out_d = nc.dram_tensor("out", [RQ, DQ], f32, kind="ExternalOutput")

    with tile.TileContext(nc) as tc:
        const_cm = tc.tile_pool(name="const", bufs=1)
        const = const_cm.__enter__()
        ones_col = const.tile([P, KV_T, 4, 1], f32)
        nc.vector.memset(ones_col[:], 1.0)
        g_row = const.tile([1, DQ], f32r)
        b_row = const.tile([1, DQ], f32r)
        nc.sync.dma_start(g_row[:], gam_d.ap())
        nc.sync.dma_start(b_row[:], bet_d.ap())
        # gamma/beta broadcast to all 128 partitions on GpSimd
        gb_bc = const.tile([P, 2, DQ], f32)
        for i, row in enumerate((g_row, b_row)):
            nc.gpsimd.partition_broadcast(gb_bc[:, i, :], row[:].bitcast(f32))

        # resident activations
        qT_sb = const.tile([P, DPO, RQ], f32r)
        kvT_sb = const.tile([P, DPO, LKV], f32r)
        for po in range(DPO):
            nc.sync.dma_start(
                qT_sb[:, po], qT_d.ap().rearrange("(po p) q -> po p q", p=P)[po])
        kv_r = kvT_d.ap().rearrange("(po p) q -> po p q", p=P)
        for po in range(DPO):
            nc.sync.dma_start(kvT_sb[:, po, 0:512], kv_r[po, :, 0:512])

        ctxT_sb = const.tile([P, N_PAIR, RQ], f32r)
        eps_t = const.tile([P, 1], f32)
        nc.vector.memset(eps_t[:], EPS)

        wq_r = wq_d.ap().rearrange("(po p) h -> p po h", p=P)
        wk_r = wk_d.ap().rearrange("(po p) h -> p po h", p=P)
        wv_r = wv_d.ap().rearrange("(po p) h -> p po h", p=P)

        with (
            tc.tile_pool(name="wpool", bufs=2) as wpool,
            tc.tile_pool(name="wvpool", bufs=1) as wvpool,
            tc.tile_pool(name="vpool", bufs=2) as vpool,
            tc.tile_pool(name="ktpool", bufs=2) as ktpool,
            tc.tile_pool(name="qtpool", bufs=2) as qtpool,
            tc.tile_pool(name="epool", bufs=2) as epool,
            tc.tile_pool(name="bpool", bufs=1) as bpool,
            tc.tile_pool(name="smpool", bufs=2) as smpool,
            tc.tile_pool(name="proj_ps", bufs=2, space="PSUM") as proj_ps,
            tc.tile_pool(name="sc_ps", bufs=2, space="PSUM") as sc_ps,
            tc.tile_pool(name="ctx_ps", bufs=2, space="PSUM") as ctx_ps,
        ):
            kv_rest_issued = False
            for hp in range(N_PAIR):
                # ---- Q^T for pair ----
                b_q = smpool.tile([P, 1], f32, tag="bias")
                nc.sync.dma_start(b_q[:], bq_d.ap()[hp])
                b_k = smpool.tile([P, 1], f32, tag="bias")
                nc.sync.dma_start(b_k[:], bk_d.ap()[hp])
                wq_blk = wpool.tile([P, DPO, P], f32r, tag="wq")
                nc.sync.dma_start(
                    wq_blk[:], wq_r[:, :, P * hp:P * (hp + 1)])
                wk_blk = wpool.tile([P, DPO, P], f32r, tag="wk")
                nc.sync.dma_start(
                    wk_blk[:], wk_r[:, :, P * hp:P * (hp + 1)])

                ps_q = proj_ps.tile([P, RQ], f32, tag="proj")
                for po in range(DPO):
                    nc.tensor.matmul(ps_q[:], wq_blk[:, po], qT_sb[:, po],
                                     start=(po == 0), stop=(po == DPO - 1))
                qt_pair = qtpool.tile([P, RQ], f32r, tag="qt")
                nc.scalar.activation(qt_pair[:], ps_q[:], AF.Identity, bias=b_q[:])

                # ---- K^T for pair (4 chunks of 512 kv cols) ----
                kt_pair = ktpool.tile([P, LKV], f32r, tag="kt")
                for c in range(4):
                    ps_k = proj_ps.tile([P, 512], f32, tag="proj")
                    for po in range(DPO):
                        nc.tensor.matmul(
                            ps_k[:], wk_blk[:, po],
                            kvT_sb[:, po, 512 * c:512 * (c + 1)],
                            start=(po == 0), stop=(po == DPO - 1))
                    nc.scalar.activation(kt_pair[:, 512 * c:512 * (c + 1)], ps_k[:],
                                         AF.Identity, bias=b_k[:])

                # ---- V projection for this quartet (every other pair) ----
                if hp % 2 == 0:
                    qt = hp // 2
                    v_sb = vpool.tile([P, KV_T, 4, HD + 1], f32r, tag="v")
                    nc.vector.tensor_copy(v_sb[:, :, :, HD:], ones_col[:])
                    wv_blk = wvpool.tile([P, DPO, 4 * HD], f32r, tag="wv")
                    nc.sync.dma_start(
                        wv_blk[:],
                        wv_r[:, :, 4 * HD * qt: 4 * HD * (qt + 1)])
                    if not kv_rest_issued:
                        kv_rest_issued = True
                        for cc in range(1, 4):
                            for po in range(DPO):
                                nc.sync.dma_start(
                                    kvT_sb[:, po, 512 * cc:512 * (cc + 1)],
                                    kv_r[po, :, 512 * cc:512 * (cc + 1)])
                    for t in range(KV_T):
                        ps_v = proj_ps.tile([P, 4 * HD], f32, tag="proj")
                        for po in range(DPO):
                            nc.tensor.matmul(
                                ps_v[:], kvT_sb[:, po, P * t:P * (t + 1)],
                                wv_blk[:, po], start=(po == 0), stop=(po == DPO - 1))
                        nc.vector.tensor_copy(
                            v_sb[:, t, :, :HD],
                            ps_v[:].rearrange("p (h d) -> p h d", h=4))

                # ---- attention for the two heads of this pair ----
                ps_c = [ctx_ps.tile([HD + 1, RQ], f32, tag="ctx", name=f"ps_c{_h}")
                        for _h in range(2)]
                for kv in range(KV_T):
                    # scores for both heads of the pair into one 2-bank psum
                    # tensor (slot h), packed via tile_position row groups
                    ps_s = sc_ps.tile([P, 2, RQ], f32, tag="sc")
                    for h in range(2):
                        lo, hi = HD * h, HD * (h + 1)
                        nc.tensor.matmul(
                            ps_s[:, h], kt_pair[lo:hi, P * kv:P * (kv + 1)],
                            qt_pair[lo:hi, :], start=True, stop=True,
                            tile_position=(HD * h, 0))
                    e_t = epool.tile([P, 2, RQ], f32r, tag="e")
                    nc.scalar.activation(e_t[:], ps_s[:], AF.Exp,
                                         scale=1.0 / np.sqrt(HD))
                    for h in range(2):
                        nc.tensor.matmul(
                            ps_c[h][:], v_sb[:, kv, (hp % 2) * 2 + h, :],
                            e_t[:, h], start=(kv == 0), stop=(kv == KV_T - 1))

                # ---- normalize ctx^T, add bv, store into ctxT_sb ----
                for h in range(2):
                    rec = smpool.tile([1, RQ], f32, tag="rec")
                    nc.vector.reciprocal(rec[:], ps_c[h][HD:HD + 1, :])
                    rec_bc = bpool.tile([HD, RQ], f32, tag="recbc")
                    nc.gpsimd.partition_broadcast(rec_bc[:], rec[:].bitcast(f32))
                    b_v = smpool.tile([HD, 1], f32, tag="bv")
                    nc.sync.dma_start(b_v[:], bv_d.ap()[2 * hp + h])
                    tmp = bpool.tile([HD, RQ], f32, tag="nrm")
                    nc.vector.tensor_tensor(tmp[:], ps_c[h][:HD, :], rec_bc[:],
                                            op=ALU.mult)
                    nc.vector.tensor_scalar(
                        ctxT_sb[HD * h:HD * (h + 1), hp, :], tmp[:], b_v[:], None,
                        op0=ALU.add)

        # ---- output projection + residual + LayerNorm ----
        with (
            tc.tile_pool(name="wo_pool", bufs=1) as wo_pool,
            tc.tile_pool(name="opool", bufs=2) as opool,
            tc.tile_pool(name="ln_sm", bufs=4) as ln_sm,
            tc.tile_pool(name="out_ps", bufs=2, space="PSUM") as out_ps,
        ):
            wo_sb = wo_pool.tile([P, DPO, DQ], f32r)
            for po in range(DPO):
                nc.sync.dma_start(
                    wo_sb[:, po],
                    wo_d.ap().rearrange("(po p) e -> po p e", p=P)[po])
            for m in range(RQ // P):
                ps_o = out_ps.tile([P, 2, 512], f32, tag="o")
                for n in range(2):
                    for po in range(DPO):
                        nc.tensor.matmul(
                            ps_o[:, n], ctxT_sb[:, po, P * m:P * (m + 1)],
                            wo_sb[:, po, 512 * n:512 * (n + 1)],
                            start=(po == 0), stop=(po == DPO - 1))
                xq_t = opool.tile([P, 2, 512], f32, tag="xq")
                nc.sync.dma_start(
                    xq_t[:],
                    xq_d.ap().rearrange("(m p) (n f) -> m p n f", p=P, f=512)[m])
                x = opool.tile([P, 2, 512], f32, tag="x")
                mu = ln_sm.tile([P, 1], f32, tag="mu")
                nc.vector.scalar_tensor_tensor(
                    x[:], ps_o[:], 1.0, xq_t[:], op0=ALU.mult, op1=ALU.add,
                    accum_out=mu[:])
                xf = x[:].rearrange("p a b -> p (a b)")
                xx = opool.tile([P, 2, 512], f32, tag="xx")
                m2 = ln_sm.tile([P, 1], f32, tag="m2")
                nc.scalar.activation(xx[:], x[:], AF.Square, accum_out=m2[:])
                nc.vector.tensor_scalar(mu[:], mu[:], 1.0 / DQ, None, op0=ALU.mult)
                musq = ln_sm.tile([P, 1], f32, tag="musq")
                nc.vector.tensor_tensor(musq[:], mu[:], mu[:], op=ALU.mult)
                var = ln_sm.tile([P, 1], f32, tag="var")
                nc.vector.tensor_scalar(var[:], m2[:], 1.0 / DQ, None, op0=ALU.mult)
                nc.vector.tensor_tensor(var[:], var[:], musq[:], op=ALU.subtract)
                sd = ln_sm.tile([P, 1], f32, tag="sd")
                nc.scalar.activation(sd[:], var[:], AF.Sqrt, bias=eps_t[:])
                rstd = ln_sm.tile([P, 1], f32, tag="rstd")
                nc.vector.reciprocal(rstd[:], sd[:])
                y = opool.tile([P, 2, 512], f32, tag="xx")
                nc.vector.scalar_tensor_tensor(
                    y[:], x[:], mu[:], gb_bc[:, 0].rearrange("p (a b) -> p a b", b=512),
                    op0=ALU.subtract, op1=ALU.mult)
                z = opool.tile([P, 2, 512], f32, tag="x")
                nc.vector.tensor_scalar(
                    z[:].rearrange("p a b -> p (a b)"),
                    y[:].rearrange("p a b -> p (a b)"), rstd[:], None, op0=ALU.mult)
                z2 = opool.tile([P, 2, 512], f32, tag="xx")
                nc.gpsimd.tensor_tensor(
                    z2[:], z[:], gb_bc[:, 1].rearrange("p (a b) -> p a b", b=512),
                    op=ALU.add)
                nc.sync.dma_start(
                    out_d.ap().rearrange("(m p) (n f) -> m p n f", p=P, f=512)[m],
                    z2[:])
        const_cm.__exit__(None, None, None)

    nc.compile()
    return nc


_NC_CACHE = None


def _get_nc():
    global _NC_CACHE
    if _NC_CACHE is None:
        _NC_CACHE = build_nc()
    return _NC_CACHE


def kernel(query, key_value, Wq, bq, Wk, bk, Wv, bv, Wo, bo, ln_gamma, ln_beta):
    query = np.asarray(query, dtype=np.float32)
    key_value = np.asarray(key_value, dtype=np.float32)
    Wq = np.ascontiguousarray(np.asarray(Wq, np.float32))
    Wk = np.ascontiguousarray(np.asarray(Wk, np.float32))
    Wv = np.ascontiguousarray(np.asarray(Wv, np.float32))
    Wo = np.ascontiguousarray(np.asarray(Wo, np.float32))
    bq = np.asarray(bq, np.float32).reshape(N_PAIR, P, 1)
    bk = np.asarray(bk, np.float32).reshape(N_PAIR, P, 1)
    bv = np.asarray(bv, np.float32).reshape(NH, HD, 1)
    gam = np.asarray(ln_gamma, np.float32).reshape(1, DQ)
    bet = np.asarray(ln_beta, np.float32).reshape(1, DQ)
    bo = np.asarray(bo, np.float32)

    nc = _get_nc()
    kvT = [np.ascontiguousarray(key_value[b].T) for b in range(B)]
    in_maps = []
    for c in range(N_CORES):
        b, rq = divmod(c, N_CORES // B)
        rows = slice(RQ * rq, RQ * (rq + 1))
        in_maps.append({
            "qT": np.ascontiguousarray(query[b, rows].T),
            "kvT": kvT[b],
            "xq": np.ascontiguousarray(query[b, rows] + bo),
            "wq": Wq, "wk": Wk, "wv": Wv, "wo": Wo,
            "bq": bq, "bk": bk, "bv": bv,
            "gamma": gam, "beta": bet,
        })
    res = run_bass_kernel_spmd(nc, in_maps, list(range(N_CORES)))
    out = np.concatenate([r["out"] for r in res.results], axis=0)
    return out.reshape(B, LQ, DQ)
